# revision 17
# baseline (speedup 1.0000x reference)
"""Trainium2 Bass kernel for a diagonal selective SSM layer.

Reference computation (per batch element b):
    alpha = sigmoid(x @ Wg.T + bg)        # (L, S)
    u     = x @ WB.T + bB                 # (L, S)
    h_t   = alpha_t * h_{t-1} + u_t       # scan over L, h in R^S
    y     = h @ WC.T + bC                 # (L, D)

Sharding: data-parallel over batch. B == 8 == n_cores, so each NeuronCore
processes exactly one batch element; the small projection weights are
replicated to every core. No collectives needed.

Per-core dataflow:
  - Gate GEMM (alpha) in fp8 e4m3 with MatmulPerfMode.DoubleRow: the PE
    packs two fp8 k-rows per cell, contracting K=256 per instruction
    (~1.5x bf16 throughput at N>=256).  Wg ships pre-scaled by 32 so its
    ~N(0, 1/32) entries use the fp8 dynamic range; the sigmoid eviction
    folds the inverse scale (out = sigmoid(psum/32 + bg)).  x ships both
    as fp8 (gate GEMM) and bf16 (input GEMM) - the extra 1B/elem of DMA
    is cheaper than an on-chip cast.
  - U GEMM and output GEMM stay bf16 (fp8 there pushes rel-err past the
    2e-2 budget; gate-only fp8 lands ~1e-2 because sigmoid's derivative
    shrinks the quantization noise ~4x).
  - All fp8 inputs (wg8 + x8) are fused host-side into ONE dram tensor
    laid out in exact consumption order, and likewise all bf16 inputs
    (wb + xb + wc).  Each dma_start costs ~650ns of issue time on its
    ring, so the kernel issues only ~5 large ordered transfers per ring
    (Sync ring: fp8 gate stream; Scalar ring: bf16 stream) with 2-8KB
    per-partition lines; compute chases the streams.
  - Recurrence: hardware linear-recurrence nc.vector.tensor_tensor_scan
    (state = a*state + u, fp32 internal state), chunk-chained via
    `initial`.
  - Output GEMM transposed (yT layout, D on partitions): the scan output
    hh (S on partitions, L free) is the moving operand, WC tiles (S
    parts, D free) the stationary one.  bias bC fuses into the PSUM
    eviction (split ScalarE/VectorE) which also casts to bf16.
  - y writebacks ride the otherwise-idle GpSimd ring.
  - HAM warm-up: a short burst of dummy matmuls bridges the engine
    preamble to first-data; the early chunks then run DMA-paced, which
    keeps ramping the PE clock gate.
  - Y GEMMs skew one chunk behind the G/U GEMMs so the PE never waits on
    the scan; the last chunks are small to shorten the tail.
"""

import numpy as np

B, L, D, S = 8, 2048, 1024, 256
P = 128
NCORES = 8
KD = D // P      # 8 k-tiles over the D contraction
KP = KD // 2     # 4 fp8 DoubleRow k-pairs
MS = S // P      # 2 partition groups over S
DT = D // P      # 8 output D-tiles

CHUNKS = [512, 512, 512, 384, 128]
OFFS = [sum(CHUNKS[:i]) for i in range(len(CHUNKS) + 1)]
YOFF = [DT * o for o in OFFS]   # y block offsets ([q][t][l] layout)
assert OFFS[-1] == L
NQ = len(CHUNKS)

MH = KD * P  # cols of one m-half of gate/input weights

# fused fp8 tensor layout (cols): wg8-m0 | x8 c0 | wg8-m1 | x8 c1..c4
WG8O = [0, MH + KD * CHUNKS[0]]
X8O = {0: MH}
_c = 2 * MH + KD * CHUNKS[0]
for _q in range(1, NQ):
    X8O[_q] = _c
    _c += KD * CHUNKS[_q]
F8_COLS = _c

# fused bf16 tensor layout (cols): wb-m0 | xb c0 | wb-m1 | xb c1 | wc | xb c2..c4
WBO = [0, MH + KD * CHUNKS[0]]
XBO = {0: MH, 1: 2 * MH + KD * CHUNKS[0]}
WCO = XBO[1] + KD * CHUNKS[1]
_c = WCO + MS * D
for _q in range(2, NQ):
    XBO[_q] = _c
    _c += KD * CHUNKS[_q]
FB_COLS = _c

WARMUP_MMS = 30  # N=128 dummy matmuls bridging preamble-end to first-data
WG_SCALE = 32.0  # Wg pre-scale before fp8 quantization (undone in eviction)

_NC_CACHE = {}


def _build_nc():
    import concourse.mybir as mybir
    import concourse.tile as tile
    from concourse import bacc

    f32 = mybir.dt.float32
    bf16 = mybir.dt.bfloat16
    f8 = mybir.dt.float8e4
    AF = mybir.ActivationFunctionType
    OP = mybir.AluOpType
    DR = mybir.MatmulPerfMode.DoubleRow

    nc = bacc.Bacc("TRN2", target_bir_lowering=False, debug=False)

    f8Q = nc.dram_tensor("f8Q", [P, F8_COLS], f8, kind="ExternalInput")
    fbQ = nc.dram_tensor("fbQ", [P, FB_COLS], bf16, kind="ExternalInput")
    bias = nc.dram_tensor("biasP", [P, 4 + DT], f32, kind="ExternalInput")
    y = nc.dram_tensor("yQ", [P, DT * L], bf16, kind="ExternalOutput")

    with tile.TileContext(nc) as tc:
        with (
            tc.tile_pool(name="persist", bufs=1) as pp,
            tc.tile_pool(name="psum", bufs=8, space="PSUM") as psp,
        ):
            f8t = pp.tile([P, F8_COLS], f8, name="f8t", tag="f8t")
            fbt = pp.tile([P, FB_COLS], bf16, name="fbt", tag="fbt")
            biast = pp.tile([P, 4 + DT], f32, name="biast", tag="biast")
            ysta = pp.tile([P, DT * L], bf16, name="ysta", tag="ysta")

            # PE warm-up fodder (no DMA dependencies)
            wul = pp.tile([P, P], bf16, name="wul", tag="wul")

            def dma8(a, b):
                nc.sync.dma_start(f8t[:, a:b], f8Q[:, a:b])

            def dmab(a, b):
                nc.sync.dma_start(fbt[:, a:b], fbQ[:, a:b])

            nc.gpsimd.memset(wul[:], 0.0)
            nc.gpsimd.dma_start(biast[:], bias[:, :])
            # One Sync-ring queue in strict global consumption order
            # (concurrent rings would dilute the head transfers); fused
            # tensors let adjacent pieces merge into ~0.4-1MB issues.
            h0 = KD * CHUNKS[0] // 2
            qtr = h0 // 2
            dma8(0, MH + h0)                       # wg8-m0 + x8 c0 kp0-1
            dma8(MH + h0, X8O[1])                  # x8 c0 kp2-3 + wg8-m1
            dmab(0, MH + qtr)                      # wb-m0 + xb c0 k0-1
            dmab(MH + qtr, MH + 3 * qtr)           # xb c0 k2-5
            dmab(MH + 3 * qtr, XBO[1])             # xb c0 k6-7 + wb-m1
            dma8(X8O[1], X8O[2])                   # x8 c1
            dmab(XBO[1], XBO[1] + h0)              # xb c1 k0-3
            dmab(XBO[1] + h0, WCO)                 # xb c1 k4-7
            dmab(WCO, WCO + MS * D)                # wc
            dma8(X8O[2], X8O[3])                   # x8 c2
            dmab(XBO[2], XBO[3])                   # xb c2
            dma8(X8O[3], F8_COLS)                  # x8 c3 + c4
            dmab(XBO[3], FB_COLS)                  # xb c3 + c4

            alpha = [pp.tile([P, L], f32, name=f"al{m}", tag=f"al{m}") for m in range(MS)]
            uu = [pp.tile([P, L], f32, name=f"uu{m}", tag=f"uu{m}") for m in range(MS)]
            hh = [pp.tile([P, L], bf16, name=f"hh{m}", tag=f"hh{m}") for m in range(MS)]

            if WARMUP_MMS:
                wps = psp.tile([P, 512], f32, name="wps", tag="ps")
                for i in range(WARMUP_MMS):
                    nc.tensor.matmul(
                        wps[:, :P], wul[:], wul[:],
                        start=(i == 0), stop=(i == WARMUP_MMS - 1),
                    )

            def emit_g(q):
                # gate GEMM: fp8 DoubleRow, K=256 per matmul
                o0, o1 = OFFS[q], OFFS[q + 1]
                cl = o1 - o0
                qs = slice(o0, o1)
                for m in range(MS):
                    ps = psp.tile([P, 512], f32, name="ps", tag="ps")
                    for kp in range(KP):
                        lhsT = f8t[:, WG8O[m] + kp * 2 * P:WG8O[m] + (kp + 1) * 2 * P
                                   ].rearrange("p (two m) -> p two m", two=2)
                        rhs = f8t[:, X8O[q] + kp * 2 * cl:X8O[q] + (kp + 1) * 2 * cl
                                  ].rearrange("p (two n) -> p two n", two=2)
                        nc.tensor.matmul(
                            ps[:, :cl], lhsT, rhs,
                            start=(kp == 0), stop=(kp == KP - 1),
                            perf_mode=DR,
                        )
                    nc.scalar.activation(
                        alpha[m][:, qs], ps[:, :cl], AF.Sigmoid,
                        bias=biast[:, m:m + 1], scale=1.0 / WG_SCALE,
                    )

            def emit_u(q):
                # input GEMM (bf16) + chunk-chained hardware scan
                o0, o1 = OFFS[q], OFFS[q + 1]
                cl = o1 - o0
                qs = slice(o0, o1)
                for m in range(MS):
                    ps = psp.tile([P, 512], f32, name="ps", tag="ps")
                    for k in range(KD):
                        nc.tensor.matmul(
                            ps[:, :cl],
                            fbt[:, WBO[m] + k * P:WBO[m] + (k + 1) * P],
                            fbt[:, XBO[q] + k * cl:XBO[q] + (k + 1) * cl],
                            start=(k == 0),
                            stop=(k == KD - 1),
                        )
                    nc.vector.tensor_scalar_add(
                        uu[m][:, qs], ps[:, :cl], biast[:, 2 + m:3 + m],
                    )
                # state = alpha*state + u
                for m in range(MS):
                    init = 0.0 if q == 0 else hh[m][:, o0 - 1:o0]
                    nc.vector.tensor_tensor_scan(
                        hh[m][:, qs], alpha[m][:, qs], uu[m][:, qs],
                        init, OP.mult, OP.add,
                    )

            def emit_y(q):
                o0, o1 = OFFS[q], OFFS[q + 1]
                cl = o1 - o0
                qs = slice(o0, o1)
                last = q == NQ - 1
                for t in range(DT):
                    ps = psp.tile([P, 512], f32, name="psy", tag="ps")
                    for m in range(MS):
                        nc.tensor.matmul(
                            ps[:, :cl],
                            fbt[:, WCO + m * D + t * P:WCO + m * D + (t + 1) * P],
                            hh[m][:, qs],
                            start=(m == 0),
                            stop=(m == MS - 1),
                        )
                    dst = ysta[:, YOFF[q] + t * cl:YOFF[q] + (t + 1) * cl]
                    bc = biast[:, 4 + t:5 + t]
                    if last and t == DT - 1:
                        # the very last eviction gates the final writeback:
                        # split it across both engines so it lands sooner
                        hl = cl // 2
                        nc.scalar.activation(
                            dst[:, :hl], ps[:, :hl], AF.Identity, bias=bc, scale=1.0
                        )
                        nc.vector.tensor_scalar_add(dst[:, hl:], ps[:, hl:cl], bc)
                    elif t % 2 == 0:
                        nc.scalar.activation(dst, ps[:, :cl], AF.Identity, bias=bc, scale=1.0)
                    else:
                        nc.vector.tensor_scalar_add(dst, ps[:, :cl], bc)
                    if t == DT // 2 - 1:
                        # first-half writeback starts while the second half
                        # of this chunk's Y GEMMs still run
                        nc.gpsimd.dma_start(
                            y[:, YOFF[q]:YOFF[q] + DT // 2 * cl],
                            ysta[:, YOFF[q]:YOFF[q] + DT // 2 * cl],
                        )
                # final chunk: second half drains on the Sync ring (idle by
                # then) so the two halves land in parallel
                eng = nc.sync if last else nc.gpsimd
                eng.dma_start(
                    y[:, YOFF[q] + DT // 2 * cl:YOFF[q + 1]],
                    ysta[:, YOFF[q] + DT // 2 * cl:YOFF[q + 1]],
                )

            # software pipeline: Y GEMMs run one chunk behind the G/U GEMMs
            # so the PE never waits on the scan.
            emit_g(0)
            emit_u(0)
            for q in range(1, NQ):
                emit_g(q)
                emit_u(q)
                emit_y(q - 1)
            emit_y(NQ - 1)

    nc.finalize()
    return nc


def _get_nc():
    if "nc" not in _NC_CACHE:
        _NC_CACHE["nc"] = _build_nc()
    return _NC_CACHE["nc"]


def _make_in_maps(x, Wg, bg, WB, bB, WC, bC):
    import ml_dtypes

    bf16 = ml_dtypes.bfloat16
    f8 = ml_dtypes.float8_e4m3
    x = np.asarray(x, dtype=np.float32)
    # fp8 gate weights, DoubleRow layout: (m, p, kp, two, j) ->
    # Wg.T[(2kp+two)*P+p, m*P+j] * WG_SCALE
    wg8P = (
        np.clip(np.asarray(Wg, dtype=np.float32).T * WG_SCALE, -240, 240)
        .astype(f8)
        .reshape(KP, 2, P, MS, P).transpose(3, 2, 0, 1, 4)
    )
    # bf16 U weights m-major: (m, p, k, j) -> WB.T[k*P+p, m*P+j]
    wbP = (
        np.asarray(WB, dtype=np.float32).T.astype(bf16)
        .reshape(KD, P, MS, P).transpose(2, 1, 0, 3)
    )
    wcP = np.asarray(WC, dtype=np.float32).T.astype(bf16).reshape(MS, P, D)

    bias = np.zeros((P, 4 + DT), dtype=np.float32)
    bias[:, 0] = np.asarray(bg, dtype=np.float32)[0:P]
    bias[:, 1] = np.asarray(bg, dtype=np.float32)[P:2 * P]
    bias[:, 2] = np.asarray(bB, dtype=np.float32)[0:P]
    bias[:, 3] = np.asarray(bB, dtype=np.float32)[P:2 * P]
    bias[:, 4:] = np.asarray(bC, dtype=np.float32).reshape(DT, P).T

    in_maps = []
    for b in range(NCORES):
        xt = np.ascontiguousarray(x[b].T)          # [D, L] f32
        xkb = xt.astype(bf16).reshape(KD, P, L)    # [k, p, l]
        xk8 = np.clip(xt, -240, 240).astype(f8).reshape(KD, P, L)

        f8q = np.empty((P, F8_COLS), dtype=f8)
        f8q[:, WG8O[0]:WG8O[0] + MH] = wg8P[0].reshape(P, MH)
        f8q[:, WG8O[1]:WG8O[1] + MH] = wg8P[1].reshape(P, MH)
        fbq = np.empty((P, FB_COLS), dtype=bf16)
        fbq[:, WBO[0]:WBO[0] + MH] = wbP[0].reshape(P, MH)
        fbq[:, WBO[1]:WBO[1] + MH] = wbP[1].reshape(P, MH)
        fbq[:, WCO:WCO + MS * D] = wcP.transpose(1, 0, 2).reshape(P, MS * D)
        for q in range(NQ):
            sl = slice(OFFS[q], OFFS[q + 1])
            cl = CHUNKS[q]
            # fp8 block: (p, kp, two, l)
            f8q[:, X8O[q]:X8O[q] + KD * cl] = (
                xk8[:, :, sl].reshape(KP, 2, P, cl).transpose(2, 0, 1, 3).reshape(P, -1)
            )
            # bf16 block: (p, k, l)
            fbq[:, XBO[q]:XBO[q] + KD * cl] = (
                xkb[:, :, sl].transpose(1, 0, 2).reshape(P, -1)
            )
        in_maps.append({
            "f8Q": f8q,
            "fbQ": fbq,
            "biasP": bias,
        })
    return in_maps


def _run(in_maps, **kwargs):
    from concourse.bass_utils import run_bass_kernel_spmd

    nc = _get_nc()
    return run_bass_kernel_spmd(nc, in_maps, list(range(NCORES)), **kwargs)


def kernel(x, Wg, bg, WB, bB, WC, bC):
    res = _run(_make_in_maps(x, Wg, bg, WB, bB, WC, bC))
    out = np.empty((NCORES, L, D), dtype=np.float32)
    for b in range(NCORES):
        yq = np.asarray(res.results[b]["yQ"])
        for q in range(NQ):
            o0, o1 = OFFS[q], OFFS[q + 1]
            cl = o1 - o0
            blk = yq[:, YOFF[q]:YOFF[q + 1]].reshape(P, DT, cl)
            # yQ[p, t, l] = y[o0+l, t*P+p]
            out[b, o0:o1, :] = blk.transpose(2, 1, 0).reshape(cl, D).astype(np.float32)
    return out


# revision 18
# speedup vs baseline: 1.0376x; 1.0376x over previous
"""Trainium2 Bass kernel for a diagonal selective SSM layer.

Reference computation (per batch element b):
    alpha = sigmoid(x @ Wg.T + bg)        # (L, S)
    u     = x @ WB.T + bB                 # (L, S)
    h_t   = alpha_t * h_{t-1} + u_t       # scan over L, h in R^S
    y     = h @ WC.T + bC                 # (L, D)

Sharding: data-parallel over batch. B == 8 == n_cores, so each NeuronCore
processes exactly one batch element; the small projection weights are
replicated to every core. No collectives needed.

Per-core dataflow:
  - Gate GEMM (alpha) in fp8 e4m3 with MatmulPerfMode.DoubleRow: the PE
    packs two fp8 k-rows per cell, contracting K=256 per instruction
    (~1.5x bf16 throughput at N>=256).  Wg ships pre-scaled by 32 so its
    ~N(0, 1/32) entries use the fp8 dynamic range; the sigmoid eviction
    folds the inverse scale (out = sigmoid(psum/32 + bg)).  x ships both
    as fp8 (gate GEMM) and bf16 (input GEMM) - the extra 1B/elem of DMA
    is cheaper than an on-chip cast.
  - U GEMM and output GEMM stay bf16 (fp8 there pushes rel-err past the
    2e-2 budget; gate-only fp8 lands ~1e-2 because sigmoid's derivative
    shrinks the quantization noise ~4x).
  - All fp8 inputs (wg8 + x8) are fused host-side into ONE dram tensor
    laid out in exact consumption order, and likewise all bf16 inputs
    (wb + xb + wc).  Each dma_start costs ~650ns of issue time on its
    ring, so the kernel issues only ~5 large ordered transfers per ring
    (Sync ring: fp8 gate stream; Scalar ring: bf16 stream) with 2-8KB
    per-partition lines; compute chases the streams.
  - Recurrence: hardware linear-recurrence nc.vector.tensor_tensor_scan
    (state = a*state + u, fp32 internal state), chunk-chained via
    `initial`.
  - Output GEMM transposed (yT layout, D on partitions): the scan output
    hh (S on partitions, L free) is the moving operand, WC tiles (S
    parts, D free) the stationary one.  bias bC fuses into the PSUM
    eviction (split ScalarE/VectorE) which also casts to bf16.
  - y writebacks ride the otherwise-idle GpSimd ring.
  - HAM warm-up: a short burst of dummy matmuls bridges the engine
    preamble to first-data; the early chunks then run DMA-paced, which
    keeps ramping the PE clock gate.
  - Y GEMMs skew one chunk behind the G/U GEMMs so the PE never waits on
    the scan; the last chunks are small to shorten the tail.
"""

import numpy as np

B, L, D, S = 8, 2048, 1024, 256
P = 128
NCORES = 8
KD = D // P      # 8 k-tiles over the D contraction
KP = KD // 2     # 4 fp8 DoubleRow k-pairs
MS = S // P      # 2 partition groups over S
DT = D // P      # 8 output D-tiles

CHUNKS = [512, 512, 512, 384, 128]
OFFS = [sum(CHUNKS[:i]) for i in range(len(CHUNKS) + 1)]
YOFF = [DT * o for o in OFFS]   # y block offsets ([q][t][l] layout)
assert OFFS[-1] == L
NQ = len(CHUNKS)

MH = KD * P  # cols of one m-half of gate/input weights

# fused fp8 tensor layout (cols): wg8-m0 | x8 c0 | wg8-m1 | x8 c1..c4
WG8O = [0, MH + KD * CHUNKS[0]]
X8O = {0: MH}
_c = 2 * MH + KD * CHUNKS[0]
for _q in range(1, NQ):
    X8O[_q] = _c
    _c += KD * CHUNKS[_q]
F8_COLS = _c

# fused bf16 tensor layout (cols): wb-m0 | xb c0 | wb-m1 | xb c1 | wc | xb c2..c4
WBO = [0, MH + KD * CHUNKS[0]]
XBO = {0: MH, 1: 2 * MH + KD * CHUNKS[0]}
WCO = XBO[1] + KD * CHUNKS[1]
_c = WCO + MS * D
for _q in range(2, NQ):
    XBO[_q] = _c
    _c += KD * CHUNKS[_q]
FB_COLS = _c

WARMUP_MMS = 30  # N=128 dummy matmuls bridging preamble-end to first-data
WG_SCALE = 32.0  # Wg pre-scale before fp8 quantization (undone in eviction)

_NC_CACHE = {}


def _build_nc():
    import concourse.mybir as mybir
    import concourse.tile as tile
    from concourse import bacc

    f32 = mybir.dt.float32
    bf16 = mybir.dt.bfloat16
    f8 = mybir.dt.float8e4
    AF = mybir.ActivationFunctionType
    OP = mybir.AluOpType
    DR = mybir.MatmulPerfMode.DoubleRow

    nc = bacc.Bacc("TRN2", target_bir_lowering=False, debug=False)

    f8Q = nc.dram_tensor("f8Q", [P, F8_COLS], f8, kind="ExternalInput")
    fbQ = nc.dram_tensor("fbQ", [P, FB_COLS], bf16, kind="ExternalInput")
    bias = nc.dram_tensor("biasP", [P, 4 + DT], f32, kind="ExternalInput")
    y = nc.dram_tensor("yQ", [P, DT * L], bf16, kind="ExternalOutput")

    with tile.TileContext(nc) as tc:
        with (
            tc.tile_pool(name="persist", bufs=1) as pp,
            tc.tile_pool(name="psum", bufs=8, space="PSUM") as psp,
        ):
            f8t = pp.tile([P, F8_COLS], f8, name="f8t", tag="f8t")
            fbt = pp.tile([P, FB_COLS], bf16, name="fbt", tag="fbt")
            biast = pp.tile([P, 4 + DT], f32, name="biast", tag="biast")
            ysta = pp.tile([P, DT * L], bf16, name="ysta", tag="ysta")

            # PE warm-up fodder (no DMA dependencies)
            wul = pp.tile([P, P], bf16, name="wul", tag="wul")

            def dma8(a, b):
                nc.sync.dma_start(f8t[:, a:b], f8Q[:, a:b])

            def dmab(a, b):
                nc.sync.dma_start(fbt[:, a:b], fbQ[:, a:b])

            nc.gpsimd.memset(wul[:], 0.0)
            nc.gpsimd.dma_start(biast[:], bias[:, :])
            # One Sync-ring queue in strict global consumption order
            # (concurrent rings would dilute the head transfers); fused
            # tensors let adjacent pieces merge into ~0.4-1MB issues.
            h0 = KD * CHUNKS[0] // 2
            qtr = h0 // 2
            dmab(0, MH + qtr)                      # wb-m0 + xb c0 k0-1
            dmab(MH + qtr, MH + 3 * qtr)           # xb c0 k2-5
            dmab(MH + 3 * qtr, XBO[1])             # xb c0 k6-7 + wb-m1
            dma8(0, MH + h0)                       # wg8-m0 + x8 c0 kp0-1
            dma8(MH + h0, X8O[1])                  # x8 c0 kp2-3 + wg8-m1
            dmab(XBO[1], XBO[1] + h0)              # xb c1 k0-3
            dmab(XBO[1] + h0, WCO)                 # xb c1 k4-7
            dma8(X8O[1], X8O[2])                   # x8 c1
            dmab(WCO, WCO + MS * D)                # wc
            dmab(XBO[2], XBO[3])                   # xb c2
            dma8(X8O[2], X8O[3])                   # x8 c2
            dmab(XBO[3], FB_COLS)                  # xb c3 + c4
            dma8(X8O[3], F8_COLS)                  # x8 c3 + c4

            alpha = [pp.tile([P, L], f32, name=f"al{m}", tag=f"al{m}") for m in range(MS)]
            uu = [pp.tile([P, L], f32, name=f"uu{m}", tag=f"uu{m}") for m in range(MS)]
            hh = [pp.tile([P, L], bf16, name=f"hh{m}", tag=f"hh{m}") for m in range(MS)]

            if WARMUP_MMS:
                wps = psp.tile([P, 512], f32, name="wps", tag="ps")
                for i in range(WARMUP_MMS):
                    nc.tensor.matmul(
                        wps[:, :P], wul[:], wul[:],
                        start=(i == 0), stop=(i == WARMUP_MMS - 1),
                    )

            def emit_g(q):
                # gate GEMM: fp8 DoubleRow, K=256 per matmul
                o0, o1 = OFFS[q], OFFS[q + 1]
                cl = o1 - o0
                qs = slice(o0, o1)
                for m in range(MS):
                    ps = psp.tile([P, 512], f32, name="ps", tag="ps")
                    for kp in range(KP):
                        lhsT = f8t[:, WG8O[m] + kp * 2 * P:WG8O[m] + (kp + 1) * 2 * P
                                   ].rearrange("p (two m) -> p two m", two=2)
                        rhs = f8t[:, X8O[q] + kp * 2 * cl:X8O[q] + (kp + 1) * 2 * cl
                                  ].rearrange("p (two n) -> p two n", two=2)
                        nc.tensor.matmul(
                            ps[:, :cl], lhsT, rhs,
                            start=(kp == 0), stop=(kp == KP - 1),
                            perf_mode=DR,
                        )
                    nc.scalar.activation(
                        alpha[m][:, qs], ps[:, :cl], AF.Sigmoid,
                        bias=biast[:, m:m + 1], scale=1.0 / WG_SCALE,
                    )
                # state = alpha*state + u (u is ready: U runs before G)
                for m in range(MS):
                    init = 0.0 if q == 0 else hh[m][:, o0 - 1:o0]
                    nc.vector.tensor_tensor_scan(
                        hh[m][:, qs], alpha[m][:, qs], uu[m][:, qs],
                        init, OP.mult, OP.add,
                    )

            def emit_u(q):
                # input GEMM (bf16) + chunk-chained hardware scan
                o0, o1 = OFFS[q], OFFS[q + 1]
                cl = o1 - o0
                qs = slice(o0, o1)
                for m in range(MS):
                    ps = psp.tile([P, 512], f32, name="ps", tag="ps")
                    for k in range(KD):
                        nc.tensor.matmul(
                            ps[:, :cl],
                            fbt[:, WBO[m] + k * P:WBO[m] + (k + 1) * P],
                            fbt[:, XBO[q] + k * cl:XBO[q] + (k + 1) * cl],
                            start=(k == 0),
                            stop=(k == KD - 1),
                        )
                    nc.vector.tensor_scalar_add(
                        uu[m][:, qs], ps[:, :cl], biast[:, 2 + m:3 + m],
                    )
            def emit_y(q):
                o0, o1 = OFFS[q], OFFS[q + 1]
                cl = o1 - o0
                qs = slice(o0, o1)
                last = q == NQ - 1
                for t in range(DT):
                    ps = psp.tile([P, 512], f32, name="psy", tag="ps")
                    for m in range(MS):
                        nc.tensor.matmul(
                            ps[:, :cl],
                            fbt[:, WCO + m * D + t * P:WCO + m * D + (t + 1) * P],
                            hh[m][:, qs],
                            start=(m == 0),
                            stop=(m == MS - 1),
                        )
                    dst = ysta[:, YOFF[q] + t * cl:YOFF[q] + (t + 1) * cl]
                    bc = biast[:, 4 + t:5 + t]
                    if last and t == DT - 1:
                        # the very last eviction gates the final writeback:
                        # split it across both engines so it lands sooner
                        hl = cl // 2
                        nc.scalar.activation(
                            dst[:, :hl], ps[:, :hl], AF.Identity, bias=bc, scale=1.0
                        )
                        nc.vector.tensor_scalar_add(dst[:, hl:], ps[:, hl:cl], bc)
                    elif t % 2 == 0:
                        nc.scalar.activation(dst, ps[:, :cl], AF.Identity, bias=bc, scale=1.0)
                    else:
                        nc.vector.tensor_scalar_add(dst, ps[:, :cl], bc)
                    if t == DT // 2 - 1:
                        # first-half writeback starts while the second half
                        # of this chunk's Y GEMMs still run
                        nc.gpsimd.dma_start(
                            y[:, YOFF[q]:YOFF[q] + DT // 2 * cl],
                            ysta[:, YOFF[q]:YOFF[q] + DT // 2 * cl],
                        )
                # final chunk: second half drains on the Sync ring (idle by
                # then) so the two halves land in parallel
                eng = nc.sync if last else nc.gpsimd
                eng.dma_start(
                    y[:, YOFF[q] + DT // 2 * cl:YOFF[q + 1]],
                    ysta[:, YOFF[q] + DT // 2 * cl:YOFF[q + 1]],
                )

            # software pipeline: Y GEMMs run one chunk behind the G/U GEMMs
            # so the PE never waits on the scan.
            emit_u(0)
            emit_g(0)
            for q in range(1, NQ):
                emit_u(q)
                emit_g(q)
                emit_y(q - 1)
            emit_y(NQ - 1)

    nc.finalize()
    return nc


def _get_nc():
    if "nc" not in _NC_CACHE:
        _NC_CACHE["nc"] = _build_nc()
    return _NC_CACHE["nc"]


def _make_in_maps(x, Wg, bg, WB, bB, WC, bC):
    import ml_dtypes

    bf16 = ml_dtypes.bfloat16
    f8 = ml_dtypes.float8_e4m3
    x = np.asarray(x, dtype=np.float32)
    # fp8 gate weights, DoubleRow layout: (m, p, kp, two, j) ->
    # Wg.T[(2kp+two)*P+p, m*P+j] * WG_SCALE
    wg8P = (
        np.clip(np.asarray(Wg, dtype=np.float32).T * WG_SCALE, -240, 240)
        .astype(f8)
        .reshape(KP, 2, P, MS, P).transpose(3, 2, 0, 1, 4)
    )
    # bf16 U weights m-major: (m, p, k, j) -> WB.T[k*P+p, m*P+j]
    wbP = (
        np.asarray(WB, dtype=np.float32).T.astype(bf16)
        .reshape(KD, P, MS, P).transpose(2, 1, 0, 3)
    )
    wcP = np.asarray(WC, dtype=np.float32).T.astype(bf16).reshape(MS, P, D)

    bias = np.zeros((P, 4 + DT), dtype=np.float32)
    bias[:, 0] = np.asarray(bg, dtype=np.float32)[0:P]
    bias[:, 1] = np.asarray(bg, dtype=np.float32)[P:2 * P]
    bias[:, 2] = np.asarray(bB, dtype=np.float32)[0:P]
    bias[:, 3] = np.asarray(bB, dtype=np.float32)[P:2 * P]
    bias[:, 4:] = np.asarray(bC, dtype=np.float32).reshape(DT, P).T

    in_maps = []
    for b in range(NCORES):
        xt = np.ascontiguousarray(x[b].T)          # [D, L] f32
        xkb = xt.astype(bf16).reshape(KD, P, L)    # [k, p, l]
        xk8 = np.clip(xt, -240, 240).astype(f8).reshape(KD, P, L)

        f8q = np.empty((P, F8_COLS), dtype=f8)
        f8q[:, WG8O[0]:WG8O[0] + MH] = wg8P[0].reshape(P, MH)
        f8q[:, WG8O[1]:WG8O[1] + MH] = wg8P[1].reshape(P, MH)
        fbq = np.empty((P, FB_COLS), dtype=bf16)
        fbq[:, WBO[0]:WBO[0] + MH] = wbP[0].reshape(P, MH)
        fbq[:, WBO[1]:WBO[1] + MH] = wbP[1].reshape(P, MH)
        fbq[:, WCO:WCO + MS * D] = wcP.transpose(1, 0, 2).reshape(P, MS * D)
        for q in range(NQ):
            sl = slice(OFFS[q], OFFS[q + 1])
            cl = CHUNKS[q]
            # fp8 block: (p, kp, two, l)
            f8q[:, X8O[q]:X8O[q] + KD * cl] = (
                xk8[:, :, sl].reshape(KP, 2, P, cl).transpose(2, 0, 1, 3).reshape(P, -1)
            )
            # bf16 block: (p, k, l)
            fbq[:, XBO[q]:XBO[q] + KD * cl] = (
                xkb[:, :, sl].transpose(1, 0, 2).reshape(P, -1)
            )
        in_maps.append({
            "f8Q": f8q,
            "fbQ": fbq,
            "biasP": bias,
        })
    return in_maps


def _run(in_maps, **kwargs):
    from concourse.bass_utils import run_bass_kernel_spmd

    nc = _get_nc()
    return run_bass_kernel_spmd(nc, in_maps, list(range(NCORES)), **kwargs)


def kernel(x, Wg, bg, WB, bB, WC, bC):
    res = _run(_make_in_maps(x, Wg, bg, WB, bB, WC, bC))
    out = np.empty((NCORES, L, D), dtype=np.float32)
    for b in range(NCORES):
        yq = np.asarray(res.results[b]["yQ"])
        for q in range(NQ):
            o0, o1 = OFFS[q], OFFS[q + 1]
            cl = o1 - o0
            blk = yq[:, YOFF[q]:YOFF[q + 1]].reshape(P, DT, cl)
            # yQ[p, t, l] = y[o0+l, t*P+p]
            out[b, o0:o1, :] = blk.transpose(2, 1, 0).reshape(cl, D).astype(np.float32)
    return out
